# revision 12
# baseline (speedup 1.0000x reference)
"""ConvNetWordEncoder Trainium2 kernel.

Computes, for a batch of words (each a sequence of L=16 character ids):
  x = emb_table[words]                          # [L, N, D] character embeddings
  y = conv1d(x, conv_w, pad=1) + b              # [N, D, L]
  out = max_t relu(y)                           # [N, D]

Algebraic core: the embedding lookup is linear, so the conv collapses into
per-tap fused tables M_k = E @ W_k^T [128 letters, 300 ch]:
  z_t = sum_k M_k^T @ onehot(letter at t+k-1)
Bias is folded into the middle tap (fires exactly once per t); relu commutes
with the temporal max.

This version targets the fp8 DoubleRow path of the PE: each fused table is
split on the host into hi = fp8(M) and lo = fp8(M - hi).  One DoubleRow
matmul computes oh^T @ hi + oh^T @ lo = oh^T @ (hi + lo) - the lo term
cancels the fp8 quantization, so per-tap error is ~1e-3 while the matmul
runs at 0.5 cycles/row (2x over fp32r).  The one-hot (exact in fp8: 0/1)
is built on the host and DMA'd in, so no engine builds it on-chip; the
same one-hot block feeds both DR slots via a stride-0 broadcast.

Per core (2048 words, 16 groups of 128), per group:
  - PE: 46 DoubleRow matmuls accumulate z_t [128w, 300c] fp32 in PSUM,
    two z per [128, 2, 512] tile (bank-aligned halves), 4 tiles cycling.
  - Drain (one PSUM operand per op; gpsimd cannot touch PSUM):
      ACT seeds 4 chains:  rm_i = relu(pair_{2i})      (PSUM -> SBUF bf16)
      DVE chains:          rm_i = max(max(pair_{2i+1}, 0), rm_i)  (fused stt)
  - Pool merges the 4 bf16 chains in SBUF; DVE folds the final pair's
    halves.  Output is stored bf16 and widened to fp32 on the host.
Drains are interleaved (ACT takes even pairs, DVE odd) so PSUM tiles free
before the PE needs them back 4 pairs later.
"""

import numpy as np
import ml_dtypes
from contextlib import ExitStack

import concourse.tile as tile
from concourse import bacc, mybir
from concourse.bass_utils import run_bass_kernel_spmd

HIDDEN = 300
NLET = 128
KSIZE = 3
L = 16
NWORDS = 16384
NCORES = 8
NW = NWORDS // NCORES          # 2048 words per core
GROUPS = NW // 128             # 16 groups of 128 words
COLS = L * 128                 # one-hot columns per group
FP32 = mybir.dt.float32
BF16 = mybir.dt.bfloat16
FP8 = mybir.dt.float8e4
DR = mybir.MatmulPerfMode.DoubleRow

_cache = {}


def _build(iters=1):
    key = iters
    if key in _cache:
        return _cache[key]
    nc = bacc.Bacc("TRN2", target_bir_lowering=False, debug=False,
                   num_devices=NCORES)

    oh_d = nc.dram_tensor("oh", [GROUPS, NLET, COLS], FP8,
                          kind="ExternalInput")
    wf_d = nc.dram_tensor("wf", [NLET, KSIZE, 2, HIDDEN], FP8,
                          kind="ExternalInput")
    out_d = nc.dram_tensor("out", [NW, HIDDEN], BF16, kind="ExternalOutput")

    Relu = mybir.ActivationFunctionType.Relu
    Max = mybir.AluOpType.max

    with tile.TileContext(nc) as tc, ExitStack() as ctx:
        const = ctx.enter_context(tc.tile_pool(name="const", bufs=1))
        ohpool = ctx.enter_context(tc.tile_pool(name="ohp", bufs=3))
        rmpool = ctx.enter_context(tc.tile_pool(name="rm", bufs=10))
        outpool = ctx.enter_context(tc.tile_pool(name="outp", bufs=4))
        pz = ctx.enter_context(tc.tile_pool(name="pz", bufs=4, space="PSUM"))

        wf = const.tile([NLET, KSIZE, 2, HIDDEN], FP8)
        nc.sync.dma_start(wf[:], wf_d.ap()[:])

        # PE p-state pre-warm: dummy DoubleRow matmuls on a memset tile (no
        # DMA dependency - starts almost immediately) keep the PE busy until
        # the first one-hot lands, so real matmuls enter close to full clock.
        wz = const.tile([NLET, 2, HIDDEN], FP8)
        nc.gpsimd.memset(wz[:], 0.0)
        warm_ps = pz.tile([128, 2, 512], FP32, tag="zp", name="warm")
        for _ in range(16):
            nc.tensor.matmul(warm_ps[:, 0, 0:HIDDEN],
                             wz[:, 0, 0:128].unsqueeze(1)
                             .broadcast_to([NLET, 2, 128]),
                             wz[:], start=True, stop=True,
                             perf_mode=DR)

        NP = L // 2  # 8 z-pairs per group
        for it in range(iters):
            for g in range(GROUPS):
                oh = ohpool.tile([NLET, COLS], FP8, tag="oh", name="oh")
                if it == 0 and g == 0:
                    # first group in 4 chunks: first matmuls gate on 1/4
                    csz = COLS // 4
                    for q in range(4):
                        nc.sync.dma_start(oh[:, q * csz:(q + 1) * csz],
                                          oh_d.ap()[g, :, q * csz:(q + 1) * csz])
                else:
                    nc.sync.dma_start(oh[:], oh_d.ap()[g])

                def ohs(s):
                    blk = oh[:, s * 128:(s + 1) * 128]
                    return blk.unsqueeze(1).broadcast_to([NLET, 2, 128])

                zp = [None] * NP

                def zt(t):
                    return zp[t // 2][:, t % 2, 0:HIDDEN]

                # drain plan (walrus rules: one PSUM operand per op, gpsimd
                # cannot touch PSUM, no tensor_tensor on gpsimd):
                #   ACT relu-seeds 5 or 6 pairs -> rm chains (bf16 SBUF)
                #   DVE stt-drains the rest onto chains (relu fused)
                #   DVE merges the chains + half-fold, all bf16 2x mode
                # a alternates 5/6 by group parity to balance ACT vs DVE.
                last = (it == iters - 1 and g == GROUPS - 1)
                if last:
                    # final group: keep the tail off ACT - p7 drains on DVE
                    # into an existing chain so only stt+fold+store trail
                    ACT_SEED = {0: 0, 2: 1, 4: 2, 6: 3}
                    DVE_CHAIN = {1: 0, 3: 1, 5: 2, 7: 0}
                elif g % 2 == 0:
                    ACT_SEED = {0: 0, 2: 1, 4: 2, 6: 3, 7: 4}
                    DVE_CHAIN = {1: 0, 3: 1, 5: 2}
                else:
                    ACT_SEED = {0: 0, 2: 1, 4: 2, 5: 3, 6: 4, 7: 5}
                    DVE_CHAIN = {1: 0, 3: 1}
                rm = [None] * 6

                def drain(j):
                    pair = zp[j][:, :, 0:HIDDEN]
                    if j in ACT_SEED:
                        r = rmpool.tile([128, 2, HIDDEN], BF16, tag="rm",
                                        name="rmseed")
                        nc.scalar.activation(r[:], pair, Relu)
                        rm[ACT_SEED[j]] = r
                    else:
                        r = rm[DVE_CHAIN[j]]
                        nc.vector.scalar_tensor_tensor(
                            r[:], pair, 0.0, r[:], Max, Max)

                for s in range(L):
                    oh_s = ohs(s)
                    if s == 0:
                        zp[0] = pz.tile([128, 2, 512], FP32, tag="zp",
                                        name="zp0")
                    if s + 1 < L:
                        if (s + 1) % 2 == 0:
                            zp[(s + 1) // 2] = pz.tile([128, 2, 512], FP32,
                                                       tag="zp", name="zpn")
                        nc.tensor.matmul(zt(s + 1), oh_s, wf[:, 0, :, :],
                                         start=True, stop=False, perf_mode=DR)
                    nc.tensor.matmul(zt(s), oh_s, wf[:, 1, :, :],
                                     start=(s == 0), stop=(s == L - 1),
                                     perf_mode=DR)
                    if s >= 1:
                        nc.tensor.matmul(zt(s - 1), oh_s, wf[:, 2, :, :],
                                         start=False, stop=True, perf_mode=DR)
                    # pair j (z_2j, z_2j+1) completes with the tap-2 matmul
                    # emitted at s == 2j+2 (pair 7 at s == 15).
                    if s >= 2 and s % 2 == 0:
                        drain((s - 2) // 2)
                drain(7)

                # bf16 merge tree on DVE (latest pairs merge last)
                if last:
                    nc.vector.tensor_tensor(rm[1][:], rm[1][:], rm[2][:], Max)
                    nc.vector.tensor_tensor(rm[1][:], rm[1][:], rm[3][:], Max)
                    nc.vector.tensor_tensor(rm[0][:], rm[0][:], rm[1][:], Max)
                else:
                    nc.vector.tensor_tensor(rm[0][:], rm[0][:], rm[1][:], Max)
                    nc.vector.tensor_tensor(rm[2][:], rm[2][:], rm[3][:], Max)
                    nc.vector.tensor_tensor(rm[0][:], rm[0][:], rm[2][:], Max)
                    if g % 2 == 0:
                        nc.vector.tensor_tensor(rm[0][:], rm[0][:], rm[4][:],
                                                Max)
                    else:
                        nc.vector.tensor_tensor(rm[4][:], rm[4][:], rm[5][:],
                                                Max)
                        nc.vector.tensor_tensor(rm[0][:], rm[0][:], rm[4][:],
                                                Max)
                outt = outpool.tile([128, HIDDEN], BF16, tag="outt")
                nc.vector.tensor_tensor(outt[:], rm[0][:, 0, :],
                                        rm[0][:, 1, :], Max)
                nc.sync.dma_start(out_d.ap()[g * 128:(g + 1) * 128, :],
                                  outt[:])

    nc.compile()
    _cache[key] = nc
    return nc


def _prep_inputs(words_batch, emb_table, conv_w, conv_b):
    emb = np.asarray(emb_table, dtype=np.float32)
    w = np.asarray(conv_w, dtype=np.float32)
    b = np.asarray(conv_b, dtype=np.float32)
    words = np.asarray(words_batch)

    # fused per-tap tables [3, 128, 300]; bias folded into the middle tap
    wfuse = np.stack([emb @ w[:, :, k].T for k in range(KSIZE)], axis=0)
    wfuse[1] += b[None, :]
    # hi/lo fp8 split: hi + lo reconstructs wfuse to ~1e-3
    hi = wfuse.astype(ml_dtypes.float8_e4m3)
    lo = (wfuse - hi.astype(np.float32)).astype(ml_dtypes.float8_e4m3)
    # [letters, tap, {hi,lo}, ch]
    wf = np.stack([np.asarray(hi), np.asarray(lo)], axis=2).transpose(1, 0, 2, 3)
    wf = np.ascontiguousarray(wf)

    ar = np.arange(NLET, dtype=words.dtype)
    in_maps = []
    for c in range(NCORES):
        wc = np.asarray(words[:, c * NW:(c + 1) * NW])       # [16, 2048]
        wg = wc.reshape(L, GROUPS, 128).transpose(1, 0, 2)   # [g, t, wi]
        # one-hot [g, letter, t*128+wi], exact 0/1 in fp8
        oh = (wg[:, None, :, :] == ar[None, :, None, None])
        oh = oh.reshape(GROUPS, NLET, COLS).astype(ml_dtypes.float8_e4m3)
        in_maps.append({"oh": oh, "wf": wf})
    return in_maps


def _run(in_maps, iters=1):
    nc = _build(iters)
    return run_bass_kernel_spmd(nc, in_maps, list(range(NCORES)),
                                trace=False)


def kernel(words_batch, emb_table, conv_w, conv_b):
    in_maps = _prep_inputs(words_batch, emb_table, conv_w, conv_b)
    res = _run(in_maps, iters=1)
    out = np.concatenate(
        [np.asarray(res.results[c]["out"]).astype(np.float32)
         for c in range(NCORES)], axis=0)
    return out


# revision 19
# speedup vs baseline: 1.0224x; 1.0224x over previous
"""ConvNetWordEncoder Trainium2 kernel.

Computes, for a batch of words (each a sequence of L=16 character ids):
  x = emb_table[words]                          # [L, N, D] character embeddings
  y = conv1d(x, conv_w, pad=1) + b              # [N, D, L]
  out = max_t relu(y)                           # [N, D]

Algebraic core: the embedding lookup is linear, so the conv collapses into
per-tap fused tables M_k = E @ W_k^T [128 letters, 300 ch]:
  z_t = sum_k M_k^T @ onehot(letter at t+k-1)
Bias is folded into the middle tap (fires exactly once per t); relu commutes
with the temporal max.

This version targets the fp8 DoubleRow path of the PE: each fused table is
split on the host into hi = fp8(M) and lo = fp8(M - hi).  One DoubleRow
matmul computes oh^T @ hi + oh^T @ lo = oh^T @ (hi + lo) - the lo term
cancels the fp8 quantization, so per-tap error is ~1e-3 while the matmul
runs at 0.5 cycles/row (2x over fp32r).  The one-hot (exact in fp8: 0/1)
is built on the host and DMA'd in, so no engine builds it on-chip; the
same one-hot block feeds both DR slots via a stride-0 broadcast.

Per core (2048 words, 16 groups of 128), per group:
  - PE: 46 DoubleRow matmuls accumulate z_t [128w, 300c] fp32 in PSUM,
    two z per [128, 2, 512] tile (bank-aligned halves), 4 tiles cycling.
  - Drain (one PSUM operand per op; gpsimd cannot touch PSUM):
      ACT seeds 4 chains:  rm_i = relu(pair_{2i})      (PSUM -> SBUF bf16)
      DVE chains:          rm_i = max(max(pair_{2i+1}, 0), rm_i)  (fused stt)
  - Pool merges the 4 bf16 chains in SBUF; DVE folds the final pair's
    halves.  Output is stored bf16 and widened to fp32 on the host.
Drains are interleaved (ACT takes even pairs, DVE odd) so PSUM tiles free
before the PE needs them back 4 pairs later.
"""

import numpy as np
import ml_dtypes
from contextlib import ExitStack

import concourse.tile as tile
from concourse import bacc, mybir
from concourse.bass_utils import run_bass_kernel_spmd

HIDDEN = 300
NLET = 128
KSIZE = 3
L = 16
NWORDS = 16384
NCORES = 8
NW = NWORDS // NCORES          # 2048 words per core
GROUPS = NW // 128             # 16 groups of 128 words
COLS = L * 128                 # one-hot columns per group
FP32 = mybir.dt.float32
BF16 = mybir.dt.bfloat16
FP8 = mybir.dt.float8e4
DR = mybir.MatmulPerfMode.DoubleRow

_cache = {}


def _build(iters=1):
    key = iters
    if key in _cache:
        return _cache[key]
    nc = bacc.Bacc("TRN2", target_bir_lowering=False, debug=False,
                   num_devices=NCORES)

    oh_d = nc.dram_tensor("oh", [GROUPS, NLET, COLS], FP8,
                          kind="ExternalInput")
    wf_d = nc.dram_tensor("wf", [NLET, KSIZE, 2, HIDDEN], FP8,
                          kind="ExternalInput")
    out_d = nc.dram_tensor("out", [NW, HIDDEN], BF16, kind="ExternalOutput")

    Relu = mybir.ActivationFunctionType.Relu
    Max = mybir.AluOpType.max

    with tile.TileContext(nc) as tc, ExitStack() as ctx:
        const = ctx.enter_context(tc.tile_pool(name="const", bufs=1))
        ohpool = ctx.enter_context(tc.tile_pool(name="ohp", bufs=4))
        rmpool = ctx.enter_context(tc.tile_pool(name="rm", bufs=10))
        outpool = ctx.enter_context(tc.tile_pool(name="outp", bufs=4))
        pz = ctx.enter_context(tc.tile_pool(name="pz", bufs=4, space="PSUM"))

        wf = const.tile([NLET, KSIZE, 2, HIDDEN], FP8)
        nc.sync.dma_start(wf[:], wf_d.ap()[:])

        # PE p-state pre-warm: dummy DoubleRow matmuls on a memset tile (no
        # DMA dependency - starts almost immediately) keep the PE busy until
        # the first one-hot lands, so real matmuls enter close to full clock.
        wz = const.tile([NLET, 2, HIDDEN], FP8)
        nc.gpsimd.memset(wz[:], 0.0)
        warm_ps = pz.tile([128, 2, 512], FP32, tag="zp", name="warm")
        for _ in range(16):
            nc.tensor.matmul(warm_ps[:, 0, 0:HIDDEN],
                             wz[:, 0, 0:128].unsqueeze(1)
                             .broadcast_to([NLET, 2, 128]),
                             wz[:], start=True, stop=True,
                             perf_mode=DR)

        NP = L // 2  # 8 z-pairs per group
        for it in range(iters):
            for g in range(GROUPS):
                oh = ohpool.tile([NLET, COLS], FP8, tag="oh", name="oh")
                if it == 0 and g == 0:
                    # first group in 4 chunks: first matmuls gate on 1/4
                    csz = COLS // 4
                    for q in range(4):
                        nc.sync.dma_start(oh[:, q * csz:(q + 1) * csz],
                                          oh_d.ap()[g, :, q * csz:(q + 1) * csz])
                else:
                    nc.sync.dma_start(oh[:], oh_d.ap()[g])

                def ohs(s):
                    blk = oh[:, s * 128:(s + 1) * 128]
                    return blk.unsqueeze(1).broadcast_to([NLET, 2, 128])

                zp = [None] * NP

                def zt(t):
                    return zp[t // 2][:, t % 2, 0:HIDDEN]

                # drain plan (walrus rules: one PSUM operand per op, gpsimd
                # cannot touch PSUM, no tensor_tensor on gpsimd):
                #   ACT relu-seeds 5 or 6 pairs -> rm chains (bf16 SBUF)
                #   DVE stt-drains the rest onto chains (relu fused)
                #   DVE merges the chains + half-fold, all bf16 2x mode
                # a alternates 5/6 by group parity to balance ACT vs DVE.
                last = False
                if g % 2 == 0:
                    ACT_SEED = {0: 0, 2: 1, 4: 2, 6: 3, 7: 4}
                    DVE_CHAIN = {1: 0, 3: 1, 5: 2}
                else:
                    ACT_SEED = {0: 0, 2: 1, 4: 2, 5: 3, 6: 4, 7: 5}
                    DVE_CHAIN = {1: 0, 3: 1}
                rm = [None] * 6

                def drain(j):
                    pair = zp[j][:, :, 0:HIDDEN]
                    if j in ACT_SEED:
                        r = rmpool.tile([128, 2, HIDDEN], BF16, tag="rm",
                                        name="rmseed")
                        nc.scalar.activation(r[:], pair, Relu)
                        rm[ACT_SEED[j]] = r
                    else:
                        r = rm[DVE_CHAIN[j]]
                        nc.vector.scalar_tensor_tensor(
                            r[:], pair, 0.0, r[:], Max, Max)

                # pair-sequential emission: all 6 matmuls of pair j are
                # contiguous (any inter-z order is legal - each z's taps
                # form their own accumulation group).  This narrows the
                # PSUM in-flight window so drains get more slack before the
                # tile is needed again 4 pairs later.
                for j in range(NP):
                    zp[j] = pz.tile([128, 2, 512], FP32, tag="zp", name="zpn")
                    for t in (2 * j, 2 * j + 1):
                        taps = [k for k in range(KSIZE)
                                if 0 <= t + k - 1 < L]
                        for i, k in enumerate(taps):
                            nc.tensor.matmul(zt(t), ohs(t + k - 1),
                                             wf[:, k, :, :],
                                             start=(i == 0),
                                             stop=(i == len(taps) - 1),
                                             perf_mode=DR)
                    if j >= 1:
                        drain(j - 1)

                # bf16 merge tree on DVE (latest pairs merge last)
                if last:
                    # tree over p0..p6 completes early; p7 chains on top
                    nc.vector.tensor_tensor(rm[0][:], rm[0][:], rm[1][:], Max)
                    nc.vector.tensor_tensor(rm[2][:], rm[2][:], rm[3][:], Max)
                    nc.vector.tensor_tensor(rm[0][:], rm[0][:], rm[2][:], Max)
                    drain(7)
                elif g % 2 == 0:
                    drain(7)
                    nc.vector.tensor_tensor(rm[0][:], rm[0][:], rm[1][:], Max)
                    nc.vector.tensor_tensor(rm[2][:], rm[2][:], rm[3][:], Max)
                    nc.vector.tensor_tensor(rm[0][:], rm[0][:], rm[2][:], Max)
                    nc.vector.tensor_tensor(rm[0][:], rm[0][:], rm[4][:], Max)
                else:
                    drain(7)
                    nc.vector.tensor_tensor(rm[0][:], rm[0][:], rm[1][:], Max)
                    nc.vector.tensor_tensor(rm[2][:], rm[2][:], rm[3][:], Max)
                    nc.vector.tensor_tensor(rm[0][:], rm[0][:], rm[2][:], Max)
                    nc.vector.tensor_tensor(rm[4][:], rm[4][:], rm[5][:], Max)
                    nc.vector.tensor_tensor(rm[0][:], rm[0][:], rm[4][:], Max)
                outt = outpool.tile([128, HIDDEN], BF16, tag="outt")
                nc.vector.tensor_tensor(outt[:], rm[0][:, 0, :],
                                        rm[0][:, 1, :], Max)
                nc.sync.dma_start(out_d.ap()[g * 128:(g + 1) * 128, :],
                                  outt[:])

    nc.compile()
    _cache[key] = nc
    return nc


def _prep_inputs(words_batch, emb_table, conv_w, conv_b):
    emb = np.asarray(emb_table, dtype=np.float32)
    w = np.asarray(conv_w, dtype=np.float32)
    b = np.asarray(conv_b, dtype=np.float32)
    words = np.asarray(words_batch)

    # fused per-tap tables [3, 128, 300]; bias folded into the middle tap
    wfuse = np.stack([emb @ w[:, :, k].T for k in range(KSIZE)], axis=0)
    wfuse[1] += b[None, :]
    # hi/lo fp8 split: hi + lo reconstructs wfuse to ~1e-3
    hi = wfuse.astype(ml_dtypes.float8_e4m3)
    lo = (wfuse - hi.astype(np.float32)).astype(ml_dtypes.float8_e4m3)
    # [letters, tap, {hi,lo}, ch]
    wf = np.stack([np.asarray(hi), np.asarray(lo)], axis=2).transpose(1, 0, 2, 3)
    wf = np.ascontiguousarray(wf)

    ar = np.arange(NLET, dtype=words.dtype)
    in_maps = []
    for c in range(NCORES):
        wc = np.asarray(words[:, c * NW:(c + 1) * NW])       # [16, 2048]
        wg = wc.reshape(L, GROUPS, 128).transpose(1, 0, 2)   # [g, t, wi]
        # one-hot [g, letter, t*128+wi], exact 0/1 in fp8
        oh = (wg[:, None, :, :] == ar[None, :, None, None])
        oh = oh.reshape(GROUPS, NLET, COLS).astype(ml_dtypes.float8_e4m3)
        in_maps.append({"oh": oh, "wf": wf})
    return in_maps


def _run(in_maps, iters=1):
    nc = _build(iters)
    return run_bass_kernel_spmd(nc, in_maps, list(range(NCORES)),
                                trace=False)


def kernel(words_batch, emb_table, conv_w, conv_b):
    in_maps = _prep_inputs(words_batch, emb_table, conv_w, conv_b)
    res = _run(in_maps, iters=1)
    out = np.concatenate(
        [np.asarray(res.results[c]["out"]).astype(np.float32)
         for c in range(NCORES)], axis=0)
    return out
